# revision 66
# baseline (speedup 1.0000x reference)
"""Multi-head self-attention TRN2 kernel (8 NeuronCores, SPMD).

Sharding: (batch 4) x (head-group 2) = 8 cores. Each core computes, for its
batch and its 4 heads, the full l=2048 attention plus a PARTIAL output
projection (rank-256 slice of the hidden dim); the host sums the two partial
y tensors of each core pair (b_out is carried by the g=0 core only).

Cost-model facts the schedule is built around: engine op cost = free-size
(columns) x cycle_t + fixed access-latency init, independent of partition
count; matmul cost = out-free-size x cycles/row (contraction and stationary
width are free; fp8 DoubleRow = 0.5 cycles/row); GPSIMD and DMA cannot touch
PSUM, so every PSUM byte drains through ACT (1.2 GHz) or DVE (0.96 GHz).
The 16.8M-element exp drain (131072 columns) is the wall: both drain
engines must run near 100% busy on maximally-wide ops.

Per-core structure:
  - K/Q proj (bf16):  ds-PAIR groups -> one [128,1024] psum tile; bias via
                      PE rank-1 matmuls so the drain is a plain full-width
                      convert (1 op, 1024 cols) on either engine; fp8 out
                      K8/Q8 [128, 2ds, 2048] (DoubleRow d-split layout).
  - V proj (bf16):    jt-PAIR groups -> [128,1024] psum tile (256-col halves
                      bank-aligned); ONE fused bias+convert tensor_tensor
                      (512 cols) -> VT4[p][128, 4h, 2jt, 65] fp8 with col 64
                      = sv (sv-scaled ones => AV col 64 accumulates sv*Z).
  - QK (fp8 DR):      one [128,1024] psum ring tile per jt (2x512-row mm);
                      ring depth 2 (banks 0-3) shared by BOTH drain engines.
  - exp:              ONE full-width op per jt tile: ACT true exp
                      [128,1024] -> fp8 (1038 ns) or DVE Schraudolph int8
                      bit-trick [128,1024] (1192 ns); greedy per-chunk
                      ACT/DVE assignment balances cumulative engine load.
  - AV (fp8 DR):      STREAMED inside the chunk: all 8 i-tiles accumulate
                      simultaneously in two persistent packed psum banks
                      (av_lo/av_hi = [128, 4, 65] each); after the exp of
                      jt-pair p lands (2-pair emission lag), 8 tiny matmuls
                      (108 ns) add pair p's contribution. Col 64 = sv*Z.
  - normalize:        per chunk boundary: TWO batched reciprocals
                      ([128,4] each) + 8 fused scale-copies -> HT bf16,
                      split across both engines by the greedy balancer.
  - transpose:        HT -> HID2 via SP-issued DMA xbar transposes (runs on
                      the idle DMA engines).
  - out proj (bf16):  ot-PAIR groups -> [128,1024] psum + b_out rank-1s;
                      ONE full-width drain; 2 DMAs to y (f32, partial).

Schedule: lead = K-pair(lc0) + Q-pair(lc0) (converts on DVE resp. ACT so
both drain engines start ~6-8 us); chunk 0 runs iq0 for all 16 jt first
(phase A) then iq1 (phase B) so it only needs those two lead groups;
remaining K/Q/V groups ride as fillers in chunk slots. PE p-state warmup
bridges the cold-clock window. Tail: last chunk's AV finishes ~0.2 us after
its last exp; norms it0-3 -> transposes -> out-proj lc2 overlap norms it4-7
-> lc3; the last drain is split so the final y DMA starts early.
"""

import sys

if '/opt/trn_rl_repo' not in sys.path:
    sys.path.insert(0, '/opt/trn_rl_repo')

import numpy as np

import concourse.bass as bass
import concourse.mybir as mybir
import bass_rust
from bass_rust import ScopedClock
from concourse.tile import TileContext
from concourse.bass_utils import run_bass_kernel_spmd

F32 = mybir.dt.float32
BF16 = mybir.dt.bfloat16
FP8 = mybir.dt.float8e4
I8 = mybir.dt.int8
EXP = mybir.ActivationFunctionType.Exp
COPY = mybir.ActivationFunctionType.Copy
IDENT_FN = mybir.ActivationFunctionType.Identity
DR = mybir.MatmulPerfMode.DoubleRow
MULT = mybir.AluOpType.mult
ADD = mybir.AluOpType.add

B, DIM, L = 4, 512, 2048
HEADS, DH = 8, 64
HID = HEADS * DH          # 512
SCALE = DH ** -0.5
GH = 4                    # heads per core (group)
GHID = GH * DH            # 256
NCT = DIM // 128          # 4 dim tiles
NJT = L // 128            # 16 key tiles
NLC = L // 512            # 4 l-chunks
SK = 2.0                  # k fp8 scale
SQ = 16.0                 # q fp8 scale (on top of SCALE)
SV = 2.0                  # v fp8 scale
EXPSCALE = 1.0 / (SK * SQ)
A8 = 8.0 / np.log(2.0) * EXPSCALE
B8 = 55.525               # tuned for round-to-nearest f32->i8 convert

NORM_V1 = False  # batched perm-AP TT (bcast_tt2-form, probe-verified)
IDENT_V1 = False  # ACT Identity+bias AP (probe-verified)

# engine cost estimates (ns) for the greedy balancer
ACT_EXP = 1038.0
DVE_EXP = 1192.0
ACT_CVT = 1038.0
DVE_CVT = 1192.0
DVE_V = 658.0
ACT_V = 612.0
ACT_NORM = 238.0
DVE_NORM = 192.0
DVE_RECIP = 129.0
ACT_O = 1038.0
DVE_O = 1192.0


def _patch_drain():
    """walrus (CoreV3) accepts at most one sem wait on the kernel-tail Drain;
    spread the end-of-kernel waits across preceding SP nops instead."""
    if getattr(TileContext, '_drain_patched', False):
        return

    def patched(self, tick_clock, wait_clock):
        nc = self.nc
        probe = nc.sync.nop()
        wait_clock.add_sem_waits(probe.ins, ScopedClock({None: tick_clock.global_clock}))
        si = probe.ins.sync_info
        waits = list(si.on_wait) if si is not None and si.on_wait else []
        if len(waits) > 1:
            si.on_wait = waits[:1]
            for w in waits[1:]:
                n = nc.sync.nop()
                nsi = n.ins.sync_info
                if nsi is None:
                    n.ins.sync_info = bass_rust.SyncInfo(on_wait=[w], on_update=[])
                else:
                    nsi.on_wait = [w]
        nc.sync.drain()
        nc.all_engine_barrier()
        popped = nc._tile_sem_poison_stack.pop()
        assert popped is self._sem_poison
        nc.clear_and_free_semaphores(list(self.sems.allocated().values()))
        nc.all_engine_barrier()

    TileContext._drain_and_barrier = patched
    TileContext._drain_patched = True


def _split_excess_waits(nc):
    """This walrus build accepts at most 1 sem wait per instruction (2 for
    EventSemaphore). Move excess waits onto injected same-engine NoOps placed
    immediately before the over-subscribed instruction."""
    ctr = 0
    for f in nc.m.functions:
        for blk in f.blocks:
            insts = list(blk.instructions)
            out = []
            changed = False
            for inst in insts:
                si = inst.sync_info
                if si is not None and si.on_wait:
                    waits = list(si.on_wait)
                    cap = 2 if isinstance(inst, bass_rust.InstEventSemaphore) else 1
                    if len(waits) > cap:
                        changed = True
                        for w in waits[:-cap]:
                            n = bass_rust.InstNoOp(name=f"waitsplit_{ctr}", ins=[], outs=[])
                            ctr += 1
                            n.engine = inst.engine
                            n.sync_info = bass_rust.SyncInfo(on_wait=[w], on_update=[])
                            out.append(n)
                        si.on_wait = waits[-cap:]
                out.append(inst)
            if changed:
                blk.instructions = out
    return nc


def build_nc(debug=False):
    _patch_drain()
    nc = bass.Bass()

    x = nc.declare_dram_parameter("x", [128, NCT, L], BF16, isOutput=False)
    wk = nc.declare_dram_parameter("wk", [128, NCT, 2, 128], BF16, isOutput=False)
    wq = nc.declare_dram_parameter("wq", [128, NCT, 2, 128], BF16, isOutput=False)
    wv = nc.declare_dram_parameter("wv", [128, NCT, GHID], BF16, isOutput=False)
    wo = nc.declare_dram_parameter("wo", [128, 2, DIM], BF16, isOutput=False)
    bvr = nc.declare_dram_parameter("bvr", [1, GHID], BF16, isOutput=False)
    bk = nc.declare_dram_parameter("bk", [128, 2], F32, isOutput=False)
    bq = nc.declare_dram_parameter("bq", [128, 2], F32, isOutput=False)
    boc = nc.declare_dram_parameter("boc", [128, NCT], F32, isOutput=False)
    ident = nc.declare_dram_parameter("ident", [128, 128], BF16, isOutput=False)
    y = nc.declare_dram_parameter("y", [DIM, L], BF16, isOutput=True)

    with TileContext(nc) as tc:
        with (
            nc.allow_low_precision(reason="fp8/bf16 matmuls; fp32 psum accumulation"),
            tc.tile_pool(name="persist", bufs=1) as persist,
            tc.tile_pool(name="expp", bufs=14) as expp,
            tc.tile_pool(name="small", bufs=2) as small,
            # PSUM (8 banks): PRIVATE rings so the drain engines never
            # couple: ACT ring 2x[128,1024] (banks 0-3, ACT exp tiles +
            # lead Q), pj ring 2x[128,512] (banks 4-5, DVE exp halves and
            # every projection/out-proj psum), av 2x[128,512] (banks 6-7,
            # persistent packed AV accumulators).
            tc.tile_pool(name="pmm", bufs=2, space="PSUM") as pmm,
        ):
            # ---- persistent SBUF tiles
            X = persist.tile([128, NCT, L], BF16, tag="x")
            WK = persist.tile([128, NCT, 2, 128], BF16, tag="wk")
            WQ = persist.tile([128, NCT, 2, 128], BF16, tag="wq")
            WV = persist.tile([128, NCT, GHID], BF16, tag="wv")
            WO = persist.tile([128, 2, DIM], BF16, tag="wo")
            BVR = persist.tile([1, GHID], BF16, tag="bvr")
            BK = persist.tile([128, 2], F32, tag="bk")
            BQ = persist.tile([128, 2], F32, tag="bq")
            BOC = persist.tile([128, NCT], F32, tag="boc")
            IDENT = persist.tile([128, 128], BF16, tag="ident")
            ONESR = persist.tile([1, 512], BF16, tag="onesr")
            ZEROR = persist.tile([1, 512], BF16, tag="zeror")
            K8 = persist.tile([128, 2, L], FP8, tag="k8")
            Q8 = persist.tile([128, 2, L], FP8, tag="q8")
            VT4 = [persist.tile([128, GH, 2, DH + 1], FP8, tag=f"vt{p}",
                                name=f"vt{p}")
                   for p in range(NJT // 2)]
            HT = persist.tile([128, NJT, GHID], BF16, tag="ht")
            HID2 = persist.tile([128, 2, L], BF16, tag="hid")

            def ch(lc):
                return slice(lc * 512, (lc + 1) * 512)

            # ---- input DMAs (order = need order)
            nc.sync.dma_start(out=WK[:], in_=wk[:, :, :, :])
            nc.sync.dma_start(out=X[:, 0, ch(0)], in_=x[:, 0, ch(0)])
            nc.sync.dma_start(out=X[:, 1, ch(0)], in_=x[:, 1, ch(0)])
            nc.sync.dma_start(out=WQ[:], in_=wq[:, :, :, :])
            nc.sync.dma_start(out=X[:, 2, ch(0)], in_=x[:, 2, ch(0)])
            nc.sync.dma_start(out=X[:, 3, ch(0)], in_=x[:, 3, ch(0)])
            nc.sync.dma_start(out=BK[:], in_=bk[:, :])
            nc.sync.dma_start(out=BQ[:], in_=bq[:, :])
            nc.sync.dma_start(out=WV[:], in_=wv[:, :, :])
            nc.sync.dma_start(out=BVR[:], in_=bvr[:, :])
            nc.sync.dma_start(out=X[:, :, ch(1)], in_=x[:, :, ch(1)])
            nc.sync.dma_start(out=X[:, :, ch(2)], in_=x[:, :, ch(2)])
            nc.sync.dma_start(out=X[:, :, ch(3)], in_=x[:, :, ch(3)])
            nc.sync.dma_start(out=WO[:], in_=wo[:, :, :])
            nc.sync.dma_start(out=BOC[:], in_=boc[:, :])
            nc.sync.dma_start(out=IDENT[:], in_=ident[:, :])
            nc.gpsimd.memset(ONESR[:], 1.0)
            nc.gpsimd.memset(ZEROR[:], 0.0)

            # ones columns of VT4 (value sv => AV col 64 = sv*Z)
            for p in range(NJT // 2):
                nc.gpsimd.memset(VT4[p][:, :, :, DH:DH + 1], SV)

            # ---- greedy ACT/DVE load balancer
            load = {'A': 0.0, 'D': 0.0}

            last_exp_eng = ['A']

            def pick(act_cost, dve_cost):
                if load['A'] + act_cost <= load['D'] + dve_cost * 1.05:
                    load['A'] += act_cost
                    return 'A'
                load['D'] += dve_cost
                return 'D'

            def pick_exp(act_cost, dve_cost, stick=350.0):
                a = load['A'] + act_cost
                d = load['D'] + dve_cost * 1.05
                if last_exp_eng[0] == 'A':
                    a -= stick
                else:
                    d -= stick
                if a <= d:
                    load['A'] += act_cost
                    last_exp_eng[0] = 'A'
                    return 'A'
                load['D'] += dve_cost
                last_exp_eng[0] = 'D'
                return 'D'

            # ---- emission helpers

            def kq_ds_q(lc, WT, KT, BT, ds, cell, nm, half, eng=None):
                """Half of one ds of a K/Q projection (2 ct matmuls); the
                second half adds the fused-bias convert."""
                if 'sl%d' % ds not in cell:
                    cell['sl%d' % ds] = pmm.tile([128, 512], F32, tag="pj",
                                                 name=f"p{nm}{lc}_{ds}")[:]
                sl = cell['sl%d' % ds]
                for ct in (0, 1) if half == 0 else (2, 3):
                    nc.tensor.matmul(sl, WT[:, ct, ds, :], X[:, ct, ch(lc)],
                                     start=(ct == 0), stop=(ct == NCT - 1))
                if half == 0:
                    return
                dst = KT[:, ds, ch(lc)]
                if eng is None:
                    e = pick(612.0, 658.0)
                else:
                    e = eng
                    load['A' if e == 'A' else 'D'] += (
                        612.0 if e == 'A' else 658.0)
                with tc.high_priority():
                    if e == 'A':
                        nc.scalar.activation(dst, sl, IDENT_FN,
                                             bias=BT[:, ds:ds + 1])
                    else:
                        nc.vector.tensor_scalar(dst, sl,
                                                BT[:, ds:ds + 1],
                                                None, ADD)

            def kq_ds(lc, WT, KT, BT, ds, cell, nm, eng=None):
                """One ds of a K/Q projection with fused-bias convert."""
                sl = pmm.tile([128, 512], F32, tag="pj",
                              name=f"p{nm}{lc}_{ds}")[:]
                for ct in range(NCT):
                    nc.tensor.matmul(sl, WT[:, ct, ds, :], X[:, ct, ch(lc)],
                                     start=(ct == 0), stop=(ct == NCT - 1))
                dst = KT[:, ds, ch(lc)]
                if IDENT_V1:
                    e = 'D'
                    load['D'] += 658.0
                elif eng is None:
                    e = pick(612.0, 658.0)
                else:
                    e = eng
                    load['A' if e == 'A' else 'D'] += (
                        612.0 if e == 'A' else 658.0)
                with tc.high_priority():
                    if e == 'A':
                        nc.scalar.activation(dst, sl, IDENT_FN,
                                             bias=BT[:, ds:ds + 1])
                    else:
                        nc.vector.tensor_scalar(dst, sl, BT[:, ds:ds + 1],
                                                None, ADD)

            def kq_parts(lc, WT, KT, BT, nm):
                cell = {}
                return [lambda: kq_ds(lc, WT, KT, BT, 0, cell, nm),
                        lambda: kq_ds(lc, WT, KT, BT, 1, cell, nm)]

            def kq_parts_q(lc, WT, KT, BT, nm):
                cell = {}
                return [lambda: kq_ds_q(lc, WT, KT, BT, 0, cell, nm, 0),
                        lambda: kq_ds_q(lc, WT, KT, BT, 0, cell, nm, 1),
                        lambda: kq_ds_q(lc, WT, KT, BT, 1, cell, nm, 0),
                        lambda: kq_ds_q(lc, WT, KT, BT, 1, cell, nm, 1)]

            def kq_pair(lc, WT, BRT, KT, nm, ps=None, eng=None, BT=None):
                """K or Q ds-pair projection for one l-chunk on the ring.
                fused mode (BT given): per-ds convert with fused bias add --
                no PE rank-1, drains start after 5 matmuls (lead).
                pair mode: bias via PE rank-1, single full-width convert."""
                if ps is None:
                    ps = pmm.tile([128, 1024], F32, tag="qk",
                                  name=f"p{nm}{lc}")
                for ds in range(2):
                    sl = ps[:, ds * 512:(ds + 1) * 512]
                    for ct in range(NCT):
                        nc.tensor.matmul(sl, WT[:, ct, ds, :], X[:, ct, ch(lc)],
                                         start=(ct == 0),
                                         stop=(ct == NCT - 1 and BT is not None))
                    if BT is None:
                        nc.tensor.matmul(sl, BRT[0:1, ds, :], ONESR[0:1, :],
                                         start=False, stop=True)
                    else:
                        dst = KT[:, ds, ch(lc)]
                        e = eng if eng is not None else pick(612.0, 658.0)
                        if eng is not None:
                            load['A' if e == 'A' else 'D'] += (
                                612.0 if e == 'A' else 658.0)
                        with tc.high_priority():
                            if e == 'A':
                                nc.scalar.activation(dst, sl, IDENT_FN,
                                                     bias=BT[:, ds:ds + 1])
                            else:
                                nc.vector.tensor_scalar(dst, sl,
                                                        BT[:, ds:ds + 1],
                                                        None, ADD)
                if BT is not None:
                    return
                dst = KT[:, :, ch(lc)]
                if eng is None:
                    eng = pick(ACT_CVT, DVE_CVT)
                elif eng == 'A':
                    load['A'] += ACT_CVT
                else:
                    load['D'] += DVE_CVT
                src = ps[:].rearrange("p (d c) -> p d c", d=2)
                with tc.high_priority():
                    if eng == 'A':
                        nc.scalar.copy(dst, src)
                    else:
                        nc.vector.tensor_copy(dst, src)

            vt_emitted = [False] * 8

            def v_half(p, k, cell):
                """One jt of V pair p: own pj tile, bias via rank-1,
                convert on the less-loaded engine at high priority."""
                ps = pmm.tile([128, 512], F32, tag="pj", name=f"pv{p}_{k}")
                jt = 2 * p + k
                sl = ps[:, 0:GHID]
                for ct in range(NCT):
                    nc.tensor.matmul(sl, X[:, ct, jt * 128:(jt + 1) * 128],
                                     WV[:, ct, :],
                                     start=(ct == 0), stop=False)
                nc.tensor.matmul(sl, ONESR[0:1, 0:128], BVR[0:1, :],
                                 start=False, stop=True)
                src = ps[:, 0:GHID].rearrange("p (h d) -> p h d", h=GH)
                dst = VT4[p][:, :, k, 0:DH]
                with tc.high_priority():
                    if pick(398.0, 392.0) == 'A':
                        nc.scalar.copy(dst, src)
                    else:
                        nc.vector.tensor_copy(dst, src)
                if k == 1:
                    vt_emitted[p] = True

            def v_parts(p):
                cell = {}
                return [lambda: v_half(p, 0, cell), lambda: v_half(p, 1, cell)]

            def exp_drain(ex, half, src_ap, eng):
                """One full-width exp: psum src -> ex[:, half, cols]."""
                if eng == 'A':
                    load['A'] += ACT_EXP
                    nc.scalar.activation(ex, src_ap, EXP, scale=EXPSCALE)
                else:
                    load['D'] += 658.0
                    nc.vector.tensor_scalar(ex.bitcast(I8), src_ap,
                                            A8, B8, MULT, ADD)

            def av_step(h, p, av_lo, av_hi, ex, its):
                """Add jt-pair p's contribution for the given i-tiles."""
                for it in its:
                    avt = av_lo if it < 4 else av_hi
                    nc.tensor.matmul(
                        avt[:, it % 4, 0:DH + 1],
                        ex[:, :, it * 128:(it + 1) * 128],
                        VT4[p][:, h, :, :],
                        start=False, stop=(p == 7), perf_mode=DR)

            def norm_block(h, ihalf, av_lo, av_hi, rt, its):
                """Batched reciprocal + ONE broadcast tensor_tensor per av
                half (4 i-tiles, 256 cols) -> HT. For the last head of an
                ihalf, also emit that ihalf's transposes (DMA for ihalf0,
                PE for the tail-critical ihalf1)."""
                if len(its) == 8:
                    # merged tail norm: one recip; lo via DVE TT, hi via ACT
                    # per-it scales; transposes lo=PE, hi=DMA per branch
                    with tc.high_priority():
                        load['D'] += 133.0
                        rin = bass.AP(tensor=av_lo.tensor, offset=av_lo.offset,
                                      ap=[list(av_lo.ap[0]), [72, 4], [1, 1]])
                        nc.vector.reciprocal(rt[0][:, 0:4],
                                             av_lo[:, :, DH:DH + 1])
                        nc.vector.reciprocal(rt[0][:, 4:8],
                                             av_hi[:, :, DH:DH + 1])
                        ht_ap = HT[:, ihalf * 8:ihalf * 8 + 4,
                                   h * DH:(h + 1) * DH]
                        ht_perm = bass.AP(
                            tensor=ht_ap.tensor, offset=ht_ap.offset,
                            ap=[list(ht_ap.ap[0]), list(ht_ap.ap[2]),
                                list(ht_ap.ap[1])])
                        av_ap = av_lo[:, :, 0:DH]
                        av_perm = bass.AP(
                            tensor=av_ap.tensor, offset=av_ap.offset,
                            ap=[list(av_ap.ap[0]), list(av_ap.ap[2]),
                                list(av_ap.ap[1])])
                        r_ap = rt[0][:, 0:4]
                        r_bc = bass.AP(tensor=r_ap.tensor, offset=r_ap.offset,
                                       ap=[list(r_ap.ap[0]), [0, DH], [1, 4]])
                        load['D'] += 392.0
                        nc.vector.tensor_tensor(ht_perm, av_perm, r_bc,
                                                op=MULT)
                        for it in (4, 5, 6, 7):
                            hs = HT[:, ihalf * 8 + it, h * DH:(h + 1) * DH]
                            load['A'] += ACT_NORM
                            nc.scalar.activation(hs, av_hi[:, it % 4, 0:DH],
                                                 COPY,
                                                 scale=rt[0][:, it:it + 1])
                    for it in (0, 1, 2, 3):
                        transp_pe(1, it)
                    for it in (4, 5):
                        transp(1, it)
                    for it in (6, 7):
                        transp_pe(1, it)
                    return
                half = its[0] // 4
                avt = av_lo if half == 0 else av_hi
                tail_hi = (h == GH - 1 and ihalf == 1)
                if tail_hi and half == 1:
                    # tail: DVE carries recip+norm-lo; give hi to ACT so the
                    # transpose copies and o drains start sooner
                    with tc.high_priority():
                        load['D'] += DVE_RECIP
                        nc.vector.reciprocal(rt[0][:, 4:8],
                                             avt[:, :, DH:DH + 1])
                        for it in its:
                            hs = HT[:, ihalf * 8 + it, h * DH:(h + 1) * DH]
                            load['A'] += ACT_NORM
                            nc.scalar.activation(hs, avt[:, it % 4, 0:DH],
                                                 COPY,
                                                 scale=rt[0][:, it:it + 1])
                    for it in its:
                        transp(1, it)
                    return
                if load['D'] > load['A'] + 1500.0:
                    # DVE backlogged: per-it ACT scales instead of the TT
                    with tc.high_priority():
                        load['D'] += DVE_RECIP
                        nc.vector.reciprocal(
                            rt[0][:, half * 4:half * 4 + 4],
                            avt[:, :, DH:DH + 1])
                        for it in its:
                            hs = HT[:, ihalf * 8 + it, h * DH:(h + 1) * DH]
                            load['A'] += ACT_NORM
                            nc.scalar.activation(hs, avt[:, it % 4, 0:DH],
                                                 COPY,
                                                 scale=rt[0][:, it:it + 1])
                    if h == GH - 1:
                        for it in its:
                            if ihalf == 0:
                                transp(0, it)
                            else:
                                transp_pe(1, it)
                    return
                if NORM_V1:
                    with tc.high_priority():
                        load['D'] += DVE_RECIP
                        nc.vector.reciprocal(
                            rt[0][:, half * 4:half * 4 + 4],
                            avt[:, :, DH:DH + 1])
                        for it in its:
                            hs = HT[:, ihalf * 8 + it, h * DH:(h + 1) * DH]
                            r1 = rt[0][:, it:it + 1]
                            if pick(ACT_NORM, DVE_NORM) == 'A':
                                nc.scalar.activation(hs, avt[:, it % 4, 0:DH],
                                                     COPY, scale=r1)
                            else:
                                nc.vector.tensor_scalar(
                                    hs, avt[:, it % 4, 0:DH], r1, None, MULT)
                    if h == GH - 1:
                        for it in its:
                            if ihalf == 0:
                                transp(0, it)
                            else:
                                transp_pe(1, it)
                    return
                load['D'] += DVE_RECIP
                ht_ap = HT[:, ihalf * 8 + half * 4:ihalf * 8 + half * 4 + 4,
                           h * DH:(h + 1) * DH]
                ht_perm = bass.AP(tensor=ht_ap.tensor, offset=ht_ap.offset,
                                  ap=[list(ht_ap.ap[0]), list(ht_ap.ap[2]),
                                      list(ht_ap.ap[1])])
                av_ap = avt[:, :, 0:DH]
                av_perm = bass.AP(tensor=av_ap.tensor, offset=av_ap.offset,
                                  ap=[list(av_ap.ap[0]), list(av_ap.ap[2]),
                                      list(av_ap.ap[1])])
                r_ap = rt[0][:, half * 4:half * 4 + 4]
                r_bc = bass.AP(tensor=r_ap.tensor, offset=r_ap.offset,
                               ap=[list(r_ap.ap[0]), [0, DH], [1, 4]])
                load['D'] += 392.0
                with tc.high_priority():
                    nc.vector.reciprocal(rt[0][:, half * 4:half * 4 + 4],
                                         avt[:, :, DH:DH + 1])
                    nc.vector.tensor_tensor(ht_perm, av_perm, r_bc, op=MULT)
                if h == GH - 1:
                    for it in its:
                        if ihalf == 0:
                            transp(0, it)
                        else:
                            transp_pe(1, it)

            def transp(ihalf, it):
                for ht in range(2):
                    dst = HID2[:, ht, (ihalf * 8 + it) * 128:(ihalf * 8 + it + 1) * 128]
                    nc.sync.dma_start_transpose(
                        dst, HT[:, ihalf * 8 + it, ht * 128:(ht + 1) * 128])

            def transp_pe(ihalf, it):
                """PE transpose (no DMA-completion sems on the critical
                chain); copies ride the less-loaded engine."""
                ps = pmm.tile([128, 1024], F32, tag="qk", name=f"ptr{it}")
                for ht in range(2):
                    ptr = ps[:, ht * 512:ht * 512 + 64].bitcast(BF16)
                    nc.tensor.transpose(
                        ptr, HT[:, ihalf * 8 + it, ht * 128:(ht + 1) * 128],
                        IDENT[:])
                    dst = HID2[:, ht, (ihalf * 8 + it) * 128:(ihalf * 8 + it + 1) * 128]
                    if pick(292.0, 258.0) == 'A':
                        nc.scalar.copy(dst, ptr)
                    else:
                        nc.vector.tensor_copy(dst, ptr)

            def o_half(pr, lc, oi, cell):
                ot = pr * 2 + oi
                sl = pmm.tile([128, 512], F32, tag="pj",
                              name=f"po{ot}_{lc}")[:]
                for ht in range(2):
                    nc.tensor.matmul(sl, WO[:, ht, ot * 128:(ot + 1) * 128],
                                     HID2[:, ht, ch(lc)],
                                     start=(ht == 0), stop=(ht == 1))
                if 'ys' not in cell:
                    cell['ys'] = small.tile([128, 2, 512], BF16, tag="ys",
                                            name=f"ys{pr}_{lc}", bufs=3)
                ys = cell['ys']
                with tc.high_priority():
                    if (not IDENT_V1) and pick(612.0, 658.0) == 'A':
                        nc.scalar.activation(ys[:, oi, :], sl, IDENT_FN,
                                             bias=BOC[:, ot:ot + 1])
                    else:
                        load['D'] += 658.0
                        nc.vector.tensor_scalar(ys[:, oi, :], sl,
                                                BOC[:, ot:ot + 1], None, ADD)
                if lc >= 2:
                    nc.sync.dma_start(out=y[ot * 128:(ot + 1) * 128, ch(lc)],
                                      in_=ys[:, oi, :])
                elif oi == 1:
                    # one DMA for both ot blocks (256 contiguous y rows)
                    ys_ap = ys[:]
                    y_ap = y[pr * 256:(pr + 1) * 256, ch(lc)]
                    y_out = bass.AP(tensor=y_ap.tensor, offset=y_ap.offset,
                                    ap=[[L, 128], [128 * L, 2], [1, 512]])
                    nc.sync.dma_start(out=y_out, in_=ys_ap)

            def o_parts(pr, lc):
                cell = {}
                return [lambda: o_half(pr, lc, 0, cell),
                        lambda: o_half(pr, lc, 1, cell)]

            def o_pair(pr, lc, fused=False):
                """Out-projection for ot pair pr at l-chunk lc. fused mode
                (tail): no rank-1; per-ot drain with fused b_out add and an
                immediate y DMA -- minimizes the last-drain chain."""
                ps = pmm.tile([128, 1024], F32, tag="qk", name=f"po{pr}_{lc}")
                for oi in range(2):
                    ot = pr * 2 + oi
                    sl = ps[:, oi * 512:(oi + 1) * 512]
                    for ht in range(2):
                        nc.tensor.matmul(sl, WO[:, ht, ot * 128:(ot + 1) * 128],
                                         HID2[:, ht, ch(lc)],
                                         start=(ht == 0),
                                         stop=(ht == 1 and fused))
                    if not fused:
                        nc.tensor.matmul(sl, BOR[0:1, ot * 128:(ot + 1) * 128],
                                         ONESR[0:1, :], start=False, stop=True)
                ys = small.tile([128, 2, 512], BF16, tag="ys",
                                name=f"ys{pr}_{lc}", bufs=4)
                if fused:
                    for oi in range(2):
                        ot = pr * 2 + oi
                        sl = ps[:, oi * 512:(oi + 1) * 512]
                        if pick(612.0, 658.0) == 'A':
                            nc.scalar.activation(ys[:, oi, :], sl, IDENT_FN,
                                                 bias=BOC[:, ot:ot + 1])
                        else:
                            nc.vector.tensor_scalar(ys[:, oi, :], sl,
                                                    BOC[:, ot:ot + 1], None, ADD)
                        nc.sync.dma_start(
                            out=y[ot * 128:(ot + 1) * 128, ch(lc)],
                            in_=ys[:, oi, :])
                else:
                    src = ps[:].rearrange("p (a b) -> p a b", a=2)
                    with tc.high_priority():
                        if pick(ACT_O, DVE_O) == 'A':
                            nc.scalar.copy(ys[:], src)
                        else:
                            nc.vector.tensor_copy(ys[:], src)
                    for oi in range(2):
                        nc.sync.dma_start(
                            out=y[(pr * 2 + oi) * 128:(pr * 2 + oi + 1) * 128, ch(lc)],
                            in_=ys[:, oi, :])

            # ---- QK emission: engine chosen first; ACT gets a private
            # [128,1024] ring tile (1 wide drain), DVE gets two pj tiles
            # (2x512 drains). Returns nothing; writes ex slices.
            def qk_tile(h, ihalf, jt_pair_or_jt, mode, ex_out):
                pass

            def qkmm(h, ihalf, out, jt, iq):
                nc.tensor.matmul(
                    out,
                    K8[h * 32:(h + 1) * 32, :, jt * 128:(jt + 1) * 128],
                    Q8[h * 32:(h + 1) * 32, :,
                       ihalf * 1024 + iq * 512:ihalf * 1024 + (iq + 1) * 512],
                    start=True, stop=True, perf_mode=DR,
                    tile_position=(h * 32, 0))

            # ---- PE p-state warmup: rank-1 ones matmuls bridge the
            # cold-clock window while the first DMAs land.
            warm = pmm.tile([128, 1024], F32, tag="qka", name="warm")
            for i in range(8):
                nc.tensor.matmul(warm[:, 0:512], ONESR[0:1, 0:128], ONESR[0:1, :],
                                 start=(i == 0), stop=(i == 7))

            # ---- lead: K(lc0) per-ds fused converts on DVE, Q(lc0) on ACT
            ck, cq = {}, {}
            kq_ds(0, WK, K8, BK, 0, ck, "k", eng='A')
            lq = pmm.tile([128, 1024], F32, tag="qka", name="leadq")
            for ds in range(2):
                sl = lq[:, ds * 512:(ds + 1) * 512]
                for ct in range(NCT):
                    nc.tensor.matmul(sl, WQ[:, ct, ds, :], X[:, ct, ch(0)],
                                     start=(ct == 0), stop=(ct == NCT - 1))
                load['D'] += 658.0
                with tc.high_priority():
                    nc.vector.tensor_scalar(Q8[:, ds, ch(0)], sl,
                                            BQ[:, ds:ds + 1], None, ADD)
                if ds == 0:
                    kq_ds(0, WK, K8, BK, 1, ck, "k", eng='A')

            # ---- chunks
            CHUNKS = [(h, ihalf) for ihalf in range(2) for h in range(GH)]

            # filler queue entries: callables, one consumed per slot
            fillq = []
            # av state carried between chunks: (h, ihalf, ex_tiles, av_lo,
            # av_hi, rt, pending list of (p, its))
            avq = []

            def pump(n=1):
                """Emit up to n 'heavy' items (norm blocks / filler parts);
                av accumulation steps are near-free on PE and flow through
                without consuming budget."""
                heavies = n
                while heavies > 0:
                    while avq and not avq[0][0] and avq[0][2][0]:
                        avq.pop(0)
                    if avq and avq[0][0]:
                        st = avq[0]
                        p, its, kind = st[0][0]
                        if kind == 'av':
                            if not vt_emitted[p]:
                                if not fillq:
                                    raise AssertionError(
                                        f"av step p={p} before v_pair")
                                fillq.pop(0)()
                                heavies -= 1
                                continue
                            st[0].pop(0)
                            h_, ihalf_, exs, alo, ahi, rt = st[1]
                            av_step(h_, p, alo, ahi, exs[p], its)
                            continue
                        st[0].pop(0)
                        h_, ihalf_, exs, alo, ahi, rt = st[1]
                        norm_block(h_, ihalf_, alo, ahi, rt, its)
                        heavies -= 1
                    elif fillq:
                        fillq.pop(0)()
                        heavies -= 1
                    else:
                        break

            def start_chunk(h, ihalf):
                av_ps = pmm.tile([128, 512], F32, tag="av", bufs=2,
                                 name=f"av{h}_{ihalf}")
                av_lo = av_ps[:, 0:288].rearrange("p (i c) -> p i c", i=4)
                av_ps2 = pmm.tile([128, 512], F32, tag="av", bufs=2,
                                  name=f"avh{h}_{ihalf}")
                av_hi = av_ps2[:, 0:288].rearrange("p (i c) -> p i c", i=4)
                # one full-slot zeroing matmul per bank: sibling packed
                # accumulators must never issue start=True themselves
                nc.tensor.matmul(av_ps[:, 0:288], ONESR[0:1, 0:128],
                                 ZEROR[0:1, 0:288], start=True, stop=False)
                nc.tensor.matmul(av_ps2[:, 0:288], ONESR[0:1, 0:128],
                                 ZEROR[0:1, 0:288], start=True, stop=False)
                rt = [small.tile([128, 8], F32, tag="r", name=f"r{h}_{ihalf}",
                                 bufs=2), set()]
                return av_lo, av_hi, rt

            # chunk 0 (h=0, ihalf=0): phase A (iq0 for all jt) then phase B.
            h0, ih0 = CHUNKS[0]
            ex_tiles0 = [expp.tile([128, 2, 1024], FP8, tag="ex",
                                   name=f"ex0_{p}") for p in range(8)]
            av_lo0, av_hi0, rt0 = start_chunk(h0, ih0)
            st0_steps = []
            st0_fin = [False]
            st0 = (st0_steps, (h0, ih0, ex_tiles0, av_lo0, av_hi0, rt0),
                   st0_fin)

            # phase A fillers (as parts): K lc1/lc2/lc3 arrive before the jt
            # that reads them (jt>=4/8/12); V pairs 0-1 pulled early for
            # drain supply; Q(lc1) before phase B.
            fillq.extend(kq_parts(1, WK, K8, BK, "k"))
            fillq.extend(v_parts(0))
            fillq.extend(kq_parts(2, WK, K8, BK, "k"))
            fillq.extend(v_parts(1))
            fillq.extend(kq_parts(3, WK, K8, BK, "k"))
            fillq.extend(kq_parts(1, WQ, Q8, BQ, "q"))
            for jp in range(8):
                eng = pick(ACT_EXP, 1316.0 + 180.0 * len(fillq))
                if eng == 'A':
                    qk = pmm.tile([128, 1024], F32, tag="qka", name=f"qa{jp}")
                    qkmm(h0, ih0, qk[:, 0:512], 2 * jp, 0)
                    pump(1)
                    qkmm(h0, ih0, qk[:, 512:1024], 2 * jp + 1, 0)
                    exp_drain(ex_tiles0[jp][:, :, 0:512], 0,
                              qk[:].rearrange("p (a b) -> p a b", a=2), 'A')
                else:
                    for k in range(2):
                        qd = pmm.tile([128, 512], F32, tag="pj",
                                      name=f"qa{jp}_{k}")
                        qkmm(h0, ih0, qd[:], 2 * jp + k, 0)
                        exp_drain(ex_tiles0[jp][:, k, 0:512], 0, qd[:], 'D')
                        if k == 0:
                            pump(1)
                pump(1)

            # phase B: iq1 for all jt; V pairs + streamed av (it0-3 free now,
            # it4-7 lag 2 pairs behind their iq1 drains).
            for p in range(2, 8):
                fillq.extend(v_parts(p))
            for jp in range(8):
                eng = pick(ACT_EXP, 1316.0 + 180.0 * len(fillq))
                if eng == 'A':
                    qk = pmm.tile([128, 1024], F32, tag="qka", name=f"qb{jp}")
                    qkmm(h0, ih0, qk[:, 0:512], 2 * jp, 1)
                    pump(1)
                    qkmm(h0, ih0, qk[:, 512:1024], 2 * jp + 1, 1)
                    exp_drain(ex_tiles0[jp][:, :, 512:1024], 1,
                              qk[:].rearrange("p (a b) -> p a b", a=2), 'A')
                else:
                    for k in range(2):
                        qd = pmm.tile([128, 512], F32, tag="pj",
                                      name=f"qb{jp}_{k}")
                        qkmm(h0, ih0, qd[:], 2 * jp + k, 1)
                        exp_drain(ex_tiles0[jp][:, k, 512:1024], 1, qd[:], 'D')
                        if k == 0:
                            pump(1)
                st0_steps.append((jp - 1, [0, 1, 2, 3], 'av')) if jp >= 1 else None
                if jp >= 3:
                    st0_steps.append((jp - 3, [4, 5, 6, 7], 'av'))
                pump(1)
            st0_steps.extend([(7, [0, 1, 2, 3], 'av'),
                              (5, [4, 5, 6, 7], 'av'), (6, [4, 5, 6, 7], 'av'),
                              (7, [4, 5, 6, 7], 'av'),
                              (None, [0, 1, 2, 3], 'norm'),
                              (None, [4, 5, 6, 7], 'norm')])
            st0_fin[0] = True

            # chunks 1-7: chunk c's av/norm work pumps during chunk c+1
            prev_st = st0
            for ci in range(1, 8):
                h, ihalf = CHUNKS[ci]
                ex_tiles = [expp.tile([128, 2, 1024], FP8, tag="ex",
                                      name=f"ex{ci}_{p}") for p in range(8)]
                av_lo, av_hi, rt = start_chunk(h, ihalf)
                steps = []
                fin = [False]
                st = (steps, (h, ihalf, ex_tiles, av_lo, av_hi, rt), fin)

                # chunk-level fillers (jt -> thunks); transposes ride inside
                # the last-head norm blocks.
                inject = {}
                if ci == 1:
                    inject[6] = kq_parts(2, WQ, Q8, BQ, "q")
                elif ci == 2:
                    inject[2] = kq_parts(3, WQ, Q8, BQ, "q")
                elif ci == 4:
                    inject[14] = o_parts(0, 0)
                elif ci == 5:
                    inject[2] = o_parts(1, 0)
                    inject[10] = o_parts(0, 1)
                elif ci == 6:
                    inject[10] = o_parts(1, 1)

                avq.append(prev_st)
                if ci == 7:
                    avq.append(st)
                for jt in range(NJT):
                    eng = pick(ACT_EXP, 1316.0 + 180.0 * len(fillq))
                    if eng == 'A':
                        qk = pmm.tile([128, 1024], F32, tag="qka",
                                      name=f"qk{ci}_{jt}")
                        qkmm(h, ihalf, qk[:, 0:512], jt, 0)
                        pump(1)
                        qkmm(h, ihalf, qk[:, 512:1024], jt, 1)
                        exp_drain(ex_tiles[jt // 2][:, jt % 2, :], jt % 2,
                                  qk[:], 'A')
                    else:
                        for iq in range(2):
                            qd = pmm.tile([128, 512], F32, tag="pj",
                                          name=f"qd{ci}_{jt}_{iq}")
                            qkmm(h, ihalf, qd[:], jt, iq)
                            exp_drain(
                                ex_tiles[jt // 2][:, jt % 2,
                                                  iq * 512:(iq + 1) * 512],
                                jt % 2, qd[:], 'D')
                            if iq == 0:
                                pump(1)
                    for th in inject.get(jt, ()):
                        fillq.append(th)
                    if jt % 2 == 1:
                        p = jt // 2
                        if p >= 2:
                            steps.append((p - 2, list(range(8)), 'av'))
                    pump(1)
                if ci == 7:
                    steps.extend([(6, [0, 1, 2, 3], 'av'),
                                  (7, [0, 1, 2, 3], 'av'),
                                  (6, [4, 5, 6, 7], 'av'),
                                  (7, [4, 5, 6, 7], 'av'),
                                  (None, list(range(8)), 'norm')])
                else:
                    steps.extend([(6, list(range(8)), 'av'),
                                  (7, [0, 1, 2, 3], 'av'),
                                  (None, [0, 1, 2, 3], 'norm'),
                                  (7, [4, 5, 6, 7], 'av'),
                                  (None, [4, 5, 6, 7], 'norm')])
                fin[0] = True
                prev_st = st

            # ---- tail: drain remaining av/norm work, then transposes and
            # the ihalf1 out-projections.
            avq.append(prev_st)
            while avq or fillq:
                pump(1)
            for pr, lc in ((0, 2), (1, 2), (0, 3)):
                for th in o_parts(pr, lc):
                    th()
            # last pair: per-ot DMAs so the final transfer chain is short
            for oi in range(2):
                ot = 2 + oi
                sl = pmm.tile([128, 512], F32, tag="pj",
                              name=f"pol{ot}")[:]
                for ht in range(2):
                    nc.tensor.matmul(sl, WO[:, ht, ot * 128:(ot + 1) * 128],
                                     HID2[:, ht, ch(3)],
                                     start=(ht == 0), stop=(ht == 1))
                ysl = small.tile([128, 512], BF16, tag="ysl",
                                 name=f"ysl{ot}", bufs=2)
                with tc.high_priority():
                    if (not IDENT_V1) and pick(612.0, 658.0) == 'A':
                        nc.scalar.activation(ysl[:], sl, IDENT_FN,
                                             bias=BOC[:, ot:ot + 1])
                    else:
                        load['D'] += 658.0
                        nc.vector.tensor_scalar(ysl[:], sl, BOC[:, ot:ot + 1],
                                                None, ADD)
                nc.sync.dma_start(out=y[ot * 128:(ot + 1) * 128, ch(3)],
                                  in_=ysl[:])

            if debug:
                dbg_specs = {
                    "dht": (HT, [128, NJT, GHID], BF16),
                    "dhid": (HID2, [128, 2, L], BF16),
                    "dk8": (K8, [128, 2, L], FP8),
                    "dq8": (Q8, [128, 2, L], FP8),
                }
                for p in range(8):
                    dbg_specs[f"dvt{p}"] = (VT4[p], [128, GH, 2, DH + 1], FP8)
                for nm, (tile, shape, dt) in dbg_specs.items():
                    d = nc.declare_dram_parameter(nm, shape, dt, isOutput=True)
                    nc.sync.dma_start(out=d[:], in_=tile[:])
    _split_excess_waits(nc)
    return nc


_NC = None


def _get_nc():
    global _NC
    if _NC is None:
        _NC = build_nc()
    return _NC


_RUNNER = None


def _get_runner():
    """Build the jitted 8-core executable once; reuse on every kernel() call."""
    global _RUNNER
    if _RUNNER is not None:
        return _RUNNER

    import jax
    from jax.sharding import Mesh, PartitionSpec
    from jax.experimental.shard_map import shard_map
    from concourse import bass2jax
    import concourse.mybir as mb

    nc = _get_nc()
    bass2jax.install_neuronx_cc_hook()

    partition_name = nc.partition_id_tensor.name if nc.partition_id_tensor else None
    in_names, out_names, out_avals, zero_outs = [], [], [], []
    for alloc in nc.m.functions[0].allocations:
        if not isinstance(alloc, mb.MemoryLocationSet):
            continue
        name = alloc.memorylocations[0].name
        if alloc.kind == "ExternalInput":
            if name != partition_name:
                in_names.append(name)
        elif alloc.kind == "ExternalOutput":
            shape = tuple(alloc.tensor_shape)
            dtype = mb.dt.np(alloc.dtype)
            out_names.append(name)
            out_avals.append(jax.core.ShapedArray(shape, dtype))
            zero_outs.append(np.zeros(shape, dtype))
    n_params = len(in_names)
    n_outs = len(out_avals)
    all_in_names = list(in_names) + list(out_names)
    if partition_name is not None:
        all_in_names.append(partition_name)

    def _body(*args):
        operands = list(args)
        if partition_name is not None:
            operands.append(bass2jax.partition_id_tensor())
        outs = bass2jax._bass_exec_p.bind(
            *operands,
            out_avals=tuple(out_avals),
            in_names=tuple(all_in_names),
            out_names=tuple(out_names),
            lowering_input_output_aliases=(),
            sim_require_finite=True,
            sim_require_nnan=True,
            nc=nc,
        )
        return tuple(outs)

    n_cores = 8
    devices = jax.devices()[:n_cores]
    assert len(devices) == n_cores, (
        f"kernel needs {n_cores} NeuronCores, found {len(jax.devices())}")
    mesh = Mesh(np.asarray(devices), ("core",))
    in_specs = (PartitionSpec("core"),) * (n_params + n_outs)
    out_specs = (PartitionSpec("core"),) * n_outs
    sharded = jax.jit(
        shard_map(_body, mesh=mesh, in_specs=in_specs, out_specs=out_specs,
                  check_rep=False),
        keep_unused=True)

    from jax.sharding import NamedSharding
    shard = NamedSharding(mesh, PartitionSpec("core"))
    dev_zeros = [
        jax.device_put(np.zeros((n_cores * z.shape[0], *z.shape[1:]), z.dtype), shard)
        for z in zero_outs
    ]
    dev_cache = {}

    def run(maps):
        import hashlib
        dev_in = []
        for nm in in_names:
            concat = np.concatenate([np.ascontiguousarray(m[nm]) for m in maps], axis=0)
            digest = hashlib.blake2b(concat.tobytes(), digest_size=16).digest()
            cached = dev_cache.get(nm)
            if cached is None or cached[0] != digest:
                cached = (digest, jax.device_put(concat, shard))
                dev_cache[nm] = cached
            dev_in.append(cached[1])
        out_arrs = sharded(*dev_in, *dev_zeros)
        return [
            {nm: np.asarray(out_arrs[i]).reshape(n_cores, *out_avals[i].shape)[c]
             for i, nm in enumerate(out_names)}
            for c in range(n_cores)
        ]

    _RUNNER = run
    return _RUNNER


def _in_maps(x, w_qkv, b_qkv, w_out, b_out):
    import ml_dtypes
    bf16 = ml_dtypes.bfloat16
    x = np.ascontiguousarray(np.asarray(x, np.float32))
    w_qkv = np.asarray(w_qkv, np.float32)
    b_qkv = np.asarray(b_qkv, np.float32)
    w_out = np.asarray(w_out, np.float32)
    b_out = np.asarray(b_out, np.float32)

    bo_pack = np.ascontiguousarray(b_out.reshape(1, DIM).astype(bf16))
    bo_zero = np.zeros_like(bo_pack)  # bias only on g=0 cores (host sums pairs)
    # d-split packing index: col p of dslice ds = head p//32, chan ds*32+p%32
    pidx = np.arange(128)
    hidx = (pidx // 32) * DH + (pidx % 32)      # [128] -> head-group hid row
    maps = []
    for c in range(8):
        b, g = c // 2, c % 2
        gh0 = g * GHID
        wkg = w_qkv[HID + gh0:HID + gh0 + GHID] * SK          # [256, 512]
        wqg = w_qkv[gh0:gh0 + GHID] * (SCALE * SQ)
        wvg = w_qkv[2 * HID + gh0:2 * HID + gh0 + GHID] * SV
        bkg = b_qkv[HID + gh0:HID + gh0 + GHID] * SK
        bqg = b_qkv[gh0:gh0 + GHID] * (SCALE * SQ)
        bvg = b_qkv[2 * HID + gh0:2 * HID + gh0 + GHID] * SV

        def pack_kq(wg):
            # -> [128 dim_p, NCT, 2 ds, 128 col]
            out = np.empty((128, NCT, 2, 128), np.float32)
            for ds in range(2):
                rows = wg[hidx + ds * 32]                      # [128, 512]
                out[:, :, ds, :] = rows.T.reshape(NCT, 128, 128).transpose(1, 0, 2)
            return np.ascontiguousarray(out.astype(bf16))

        maps.append({
            "x": np.ascontiguousarray(
                x[b].reshape(NCT, 128, L).transpose(1, 0, 2).astype(bf16)),
            "wk": pack_kq(wkg),
            "wq": pack_kq(wqg),
            "wv": np.ascontiguousarray(
                wvg.T.reshape(NCT, 128, GHID).transpose(1, 0, 2).astype(bf16)),
            "wo": np.ascontiguousarray(
                w_out.T[gh0:gh0 + GHID].reshape(2, 128, DIM)
                .transpose(1, 0, 2).astype(bf16)),
            "bk": np.ascontiguousarray(
                np.stack([bkg[hidx], bkg[hidx + 32]], axis=1)),
            "bq": np.ascontiguousarray(
                np.stack([bqg[hidx], bqg[hidx + 32]], axis=1)),
            "bkr": np.ascontiguousarray(
                np.stack([bkg[hidx], bkg[hidx + 32]])[None].astype(bf16)),
            "bqr": np.ascontiguousarray(
                np.stack([bqg[hidx], bqg[hidx + 32]])[None].astype(bf16)),
            "boc": (np.ascontiguousarray(b_out.reshape(NCT, 128).T)
                    if g == 0 else np.zeros((128, NCT), np.float32)),
            "bvr": np.ascontiguousarray(bvg.reshape(1, GHID).astype(bf16)),
            "bo": bo_pack if g == 0 else bo_zero,
            "ident": np.eye(128, dtype=bf16),
        })
    return maps


def kernel(x, w_qkv, b_qkv, w_out, b_out):
    maps = _in_maps(x, w_qkv, b_qkv, w_out, b_out)
    results = _get_runner()(maps)
    out = np.empty((B, DIM, L), np.float32)
    for b in range(B):
        out[b] = (results[2 * b]["y"].astype(np.float32)
                  + results[2 * b + 1]["y"].astype(np.float32))
    return out
